# Initial kernel scaffold
#
"""Trainium2 Bass kernel for the soft decision-tree MoE layer.

Math: with q_j = sigmoid(x . dec_w[7+j] + dec_b[7+j]) for j=0..7 (only the
last level of decision nodes feeds the leaves), the reference output is

    y = sum_l p_l * (x @ W_l^T + b_l),   p_{2j} = q_j, p_{2j+1} = 1 - q_j

which collapses to 9 GEMMs instead of 16:

    y = x @ W_base^T + sum_j (q_j * x) @ dW_j^T + Baug^T @ [q; 1]

    W_base = sum_j W_{2j+1},  dW_j = W_{2j} - W_{2j+1}
    Baug rows 0..7 = b_{2j} - b_{2j+1}, row 8 = sum_j b_{2j+1}

All 9 GEMMs + the bias GEMM accumulate in PSUM (out^T layout: outputs on
partitions, rows on the moving free dim). The per-row scale q_j * x is done
on the VectorE against a PE-broadcast copy of q_j. Matmuls run in float32r
(fp22 multiply, fp32 accumulate) which streams at full PE rate for free
dim >= 256.

Sharding over 8 cores: 4 row groups (1024 rows each) x 2 output halves
(512 outs each). No cross-core communication; host assembles the slabs.
"""

import numpy as np

import concourse.bass as bass
import concourse.tile as tile
from concourse import mybir
from concourse.alu_op_type import AluOpType
from concourse.bass_utils import run_bass_kernel_spmd

f32 = mybir.dt.float32
f32r = mybir.dt.float32r

B, S, D = 2, 2048, 1024
R = B * S                  # 4096 rows total
NJ = 9                     # W_base + 8 deltas
NDC = D // 128             # 8 contraction chunks
N_ROW_GROUPS = 4
N_O_HALVES = 2
N_CORES = N_ROW_GROUPS * N_O_HALVES
R_CORE = R // N_ROW_GROUPS         # 1024 rows per core
O_CORE = D // N_O_HALVES           # 512 outputs per core
NOC = O_CORE // 128                # 4 output chunks per core
NB = 512                           # moving-block (fp32 max free dim)
NRB = R_CORE // NB                 # 2 row blocks per core

# run options that test.py may override
RUN_KWARGS = {}
LAST_RESULTS = None

_BUILD_CACHE = {}


def _build_nc():
    nc = bass.Bass()

    xt_d = nc.dram_tensor("xt", [128, NDC, R_CORE], f32, kind="ExternalInput")
    wt_d = nc.dram_tensor("wt", [NJ, NOC, 128, NDC, 128], f32, kind="ExternalInput")
    dwt_d = nc.dram_tensor("dwt", [128, NDC, 8], f32, kind="ExternalInput")
    db_d = nc.dram_tensor("db", [8, 1], f32, kind="ExternalInput")
    baug_d = nc.dram_tensor("baug", [NOC, 9, 128], f32, kind="ExternalInput")
    eye_d = nc.dram_tensor("eye8", [8, 8, 128], f32, kind="ExternalInput")
    ones_d = nc.dram_tensor("ones", [1, R_CORE], f32, kind="ExternalInput")
    out_d = nc.dram_tensor("out", [NOC, 128, R_CORE], f32, kind="ExternalOutput")

    with tile.TileContext(nc) as tc:
        with (
            tc.tile_pool(name="const", bufs=1) as constp,
            tc.tile_pool(name="xsp", bufs=2) as xsp,
            tc.tile_pool(name="wp", bufs=6) as wp,
            tc.tile_pool(name="ostp", bufs=2) as ostp,
            tc.tile_pool(name="psp", bufs=1, space="PSUM") as psp,
        ):
            xt_sb = constp.tile([128, NDC, R_CORE], f32, tag="xt", name="xt_sb")
            qb_sb = constp.tile([128, 8, R_CORE], f32, tag="qb", name="qb_sb")
            qaug_sb = constp.tile([9, R_CORE], f32, tag="qaug", name="qaug_sb")
            dwt_sb = constp.tile([128, NDC, 8], f32, tag="dwt", name="dwt_sb")
            db_sb = constp.tile([8, 1], f32, tag="db", name="db_sb")
            baug_sb = constp.tile([9, NOC, 128], f32, tag="baug", name="baug_sb")
            eye_sb = constp.tile([8, 8, 128], f32, tag="eye", name="eye_sb")

            # input DMAs (per-partition rows are contiguous in the host layout)
            for c in range(NDC):
                nc.sync.dma_start(out=xt_sb[:, c, :], in_=xt_d[:, c, :])
            nc.sync.dma_start(out=dwt_sb[:, :, :], in_=dwt_d[:, :, :])
            nc.sync.dma_start(out=db_sb[:, :], in_=db_d[:, :])
            nc.sync.dma_start(out=baug_sb[:, :, :], in_=baug_d.rearrange("o k p -> k o p"))
            nc.sync.dma_start(out=eye_sb[:, :, :], in_=eye_d[:, :, :])
            nc.sync.dma_start(out=qaug_sb[8:9, :], in_=ones_d[:, :])

            # 8 PSUM accumulator banks: out^T[oc*128:(oc+1)*128, rb*512:(rb+1)*512]
            acc = [
                [psp.tile([128, NB], f32, tag=f"acc{oc}{rb}", name=f"acc{oc}{rb}") for rb in range(NRB)]
                for oc in range(NOC)
            ]

            # --- phase A1: decision GEMM + sigmoid -> qaug rows 0..7 ---
            for rb in range(NRB):
                dec_ps = acc[0][rb]
                for c in range(NDC):
                    nc.tensor.matmul(
                        dec_ps[0:8, :],
                        dwt_sb[:, c, :].bitcast(f32r),
                        xt_sb[:, c, bass.ts(rb, NB)].bitcast(f32r),
                        start=(c == 0),
                        stop=(c == NDC - 1),
                    )
                nc.scalar.activation(
                    qaug_sb[0:8, bass.ts(rb, NB)],
                    dec_ps[0:8, :],
                    mybir.ActivationFunctionType.Sigmoid,
                    bias=db_sb[0:8, 0:1],
                    scale=1.0,
                )

            # --- phase B j=0: base GEMM (no scaling needed) ---
            def main_mms(j, src):
                for oc in range(NOC):
                    w_sb = wp.tile([128, NDC, 128], f32, tag="w", name="w_sb")
                    nc.sync.dma_start(out=w_sb[:, :, :], in_=wt_d[j, oc])
                    for rb in range(NRB):
                        for c in range(NDC):
                            nc.tensor.matmul(
                                acc[oc][rb][:, :],
                                w_sb[:, c, :].bitcast(f32r),
                                src[:, c, bass.ts(rb, NB)].bitcast(f32r),
                                start=(j == 0 and c == 0),
                                stop=False,
                            )
                    if j == NJ - 1:
                        # bias GEMM closes the accumulation; drain + store
                        ost = ostp.tile([128, R_CORE], f32, tag="ost", name="ost")
                        for rb in range(NRB):
                            nc.tensor.matmul(
                                acc[oc][rb][:, :],
                                baug_sb[:, oc, :].bitcast(f32r),
                                qaug_sb[:, bass.ts(rb, NB)].bitcast(f32r),
                                start=False,
                                stop=True,
                            )
                            nc.vector.tensor_copy(ost[:, bass.ts(rb, NB)], acc[oc][rb][:, :])
                        nc.sync.dma_start(out=out_d[oc], in_=ost[:, :])

            main_mms(0, xt_sb)

            # --- phase A2: broadcast q_j across partitions via PE outer product ---
            # scratch banks: acc[1..3][*] (their j=0 accumulation restarts via
            # start=True? no -- j=0 already wrote them). Use dedicated ordering:
            # the broadcasts run between j=0 and j=1 on the PE; they must not
            # touch banks that hold live j=0 partial sums. So broadcast into
            # the dec banks is also wrong. Instead: PE broadcasts write to
            # acc banks BEFORE j=0? -- resolved by doing broadcasts right
            # after the dec phase, before main_mms(0) touches those banks.
            # (See emission order fixup below.)

            for j in range(1, NJ):
                xs_sb = xsp.tile([128, NDC, R_CORE], f32, tag="xs", name="xs_sb")
                for c in range(NDC):
                    nc.vector.tensor_tensor(
                        xs_sb[:, c, :],
                        xt_sb[:, c, :],
                        qb_sb[:, j - 1, :],
                        AluOpType.mult,
                    )
                main_mms(j, xs_sb)

    return nc


def _get_nc():
    if "nc" not in _BUILD_CACHE:
        _BUILD_CACHE["nc"] = _build_nc()
    return _BUILD_CACHE["nc"]


def _prep_shared(dec_w, dec_b, leaf_w, leaf_b):
    leaf_w = np.asarray(leaf_w, np.float32)
    leaf_b = np.asarray(leaf_b, np.float32)
    dec_w = np.asarray(dec_w, np.float32)
    dec_b = np.asarray(dec_b, np.float32)

    w_odd = leaf_w[1::2]                         # [8, o, d]
    wcat = np.empty((NJ, D, D), np.float32)      # [j, o, d]
    wcat[0] = w_odd.sum(0)
    wcat[1:] = leaf_w[0::2] - w_odd
    wt_full = wcat.transpose(0, 2, 1)            # [j, d, o]

    # per o-half tiled weights: [NJ, NOC, 128(p=o), NDC(c), 128(d rows of chunk)]
    # stationary tile for (j, oc, c) must be [d-chunk(128 partitions), o(128)]
    wt_halves = []
    for oh in range(N_O_HALVES):
        blk = wt_full[:, :, oh * O_CORE:(oh + 1) * O_CORE]      # [j, 1024(d), 512(o)]
        blk = blk.reshape(NJ, NDC, 128, NOC, 128)               # [j, c, dp, oc, o]
        blk = blk.transpose(0, 3, 2, 1, 4)                      # [j, oc, dp, c, o]
        wt_halves.append(np.ascontiguousarray(blk))

    dwt = np.ascontiguousarray(dec_w[7:15].T.reshape(NDC, 128, 8).transpose(1, 0, 2))
    db = np.ascontiguousarray(dec_b[7:15].reshape(8, 1))

    b_odd = leaf_b[1::2]
    baug = np.empty((9, D), np.float32)
    baug[0:8] = leaf_b[0::2] - b_odd
    baug[8] = b_odd.sum(0)
    baug_halves = [
        np.ascontiguousarray(
            baug[:, oh * O_CORE:(oh + 1) * O_CORE].reshape(9, NOC, 128).transpose(1, 0, 2)
        )
        for oh in range(N_O_HALVES)
    ]

    eye8 = np.zeros((8, 8, 128), np.float32)
    eye8[np.arange(8), np.arange(8), :] = 1.0
    ones = np.ones((1, R_CORE), np.float32)
    return wt_halves, dwt, db, baug_halves, eye8, ones


def kernel(x, dec_w, dec_b, leaf_w, leaf_b):
    global LAST_RESULTS
    x = np.asarray(x, np.float32)
    wt_halves, dwt, db, baug_halves, eye8, ones = _prep_shared(dec_w, dec_b, leaf_w, leaf_b)

    xt = np.ascontiguousarray(x.reshape(R, D).T)    # [1024(d), 4096(r)]

    in_maps = []
    for core in range(N_CORES):
        rg, oh = divmod(core, N_O_HALVES)
        xt_core = np.ascontiguousarray(
            xt[:, rg * R_CORE:(rg + 1) * R_CORE]
            .reshape(NDC, 128, R_CORE)
            .transpose(1, 0, 2)
        )
        in_maps.append({
            "xt": xt_core,
            "wt": wt_halves[oh],
            "dwt": dwt,
            "db": db,
            "baug": baug_halves[oh],
            "eye8": eye8,
            "ones": ones,
        })

    nc = _get_nc()
    res = run_bass_kernel_spmd(nc, in_maps, core_ids=list(range(N_CORES)), **RUN_KWARGS)
    LAST_RESULTS = res

    out_t = np.empty((D, R), np.float32)
    for core in range(N_CORES):
        rg, oh = divmod(core, N_O_HALVES)
        o = res.results[core]["out"]      # [NOC, 128, R_CORE]
        for oc in range(NOC):
            out_t[oh * O_CORE + oc * 128: oh * O_CORE + (oc + 1) * 128,
                  rg * R_CORE:(rg + 1) * R_CORE] = o[oc]
    return np.ascontiguousarray(out_t.T).reshape(B, S, D)


# revision 6
# speedup vs baseline: 1.0145x; 1.0145x over previous
"""Trainium2 Bass kernel for the soft decision-tree MoE layer.

Math: with q_j = sigmoid(x . dec_w[7+j] + dec_b[7+j]) for j=0..7 (only the
last level of decision nodes feeds the leaves), the reference output is

    y = sum_l p_l * (x @ W_l^T + b_l),   p_{2j} = q_j, p_{2j+1} = 1 - q_j

which collapses to 9 GEMMs instead of 16:

    y = x @ W_base^T + sum_j (q_j * x) @ dW_j^T + Baug^T @ [q; 1]

    W_base = sum_j W_{2j+1},  dW_j = W_{2j} - W_{2j+1}
    Baug rows 0..7 = b_{2j} - b_{2j+1}, row 8 = sum_j b_{2j+1}

All 9 GEMMs + the bias GEMM accumulate in PSUM (out^T layout: outputs on
partitions, rows on the moving free dim). The per-row scale q_j * x is done
on the VectorE against a PE-broadcast copy of q_j (outer product with a
one-hot stationary operand, since engines cannot read partition-offset
APs). Matmuls run in float32r (fp22 multiply, fp32 accumulate), which
streams at full PE rate for free dim >= 256.

Sharding over 8 cores: 4 row groups (1024 rows each) x 2 output halves
(512 outs each). No cross-core communication; host assembles the slabs.
"""

import numpy as np

import concourse.bass as bass
import concourse.bacc as bacc
import concourse.tile as tile
from concourse import mybir
from concourse.alu_op_type import AluOpType
from concourse.bass_utils import run_bass_kernel_spmd

f32 = mybir.dt.float32
f32r = mybir.dt.float32r

B, S, D = 2, 2048, 1024
R = B * S                  # 4096 rows total
NJ = 9                     # W_base + 8 deltas
NDC = D // 128             # 8 contraction chunks
N_ROW_GROUPS = 4
N_O_HALVES = 2
N_CORES = N_ROW_GROUPS * N_O_HALVES
R_CORE = R // N_ROW_GROUPS         # 1024 rows per core
O_CORE = D // N_O_HALVES           # 512 outputs per core
NOC = O_CORE // 128                # 4 output chunks per core
NB = 512                           # moving-block (fp32 max free dim)
NRB = R_CORE // NB                 # 2 row blocks per core

# run options that test.py may override (e.g. trace=True)
RUN_KWARGS = {}
LAST_RESULTS = None

_BUILD_CACHE = {}


def _build_nc():
    nc = bacc.Bacc(None)

    xt_d = nc.dram_tensor("xt", [128, NDC, R_CORE], f32r, kind="ExternalInput")
    wt_d = nc.dram_tensor("wt", [NJ, NOC, 128, NDC, 128], f32r, kind="ExternalInput")
    dwt_d = nc.dram_tensor("dwt", [128, NDC, 8], f32r, kind="ExternalInput")
    db_d = nc.dram_tensor("db", [8, 1], f32, kind="ExternalInput")
    baug_d = nc.dram_tensor("baug", [9, NOC, 128], f32r, kind="ExternalInput")
    eye_d = nc.dram_tensor("eye8", [8, 8, 128], f32r, kind="ExternalInput")
    ones_d = nc.dram_tensor("ones", [1, R_CORE], f32r, kind="ExternalInput")
    out_d = nc.dram_tensor("out", [NOC, 128, R_CORE], f32, kind="ExternalOutput")

    with tile.TileContext(nc) as tc:
        with (
            tc.tile_pool(name="const", bufs=1) as constp,
            tc.tile_pool(name="xsp", bufs=2) as xsp,
            tc.tile_pool(name="wp", bufs=8) as wp,
            tc.tile_pool(name="ostp", bufs=2) as ostp,
            tc.tile_pool(name="psp", bufs=1, space="PSUM") as psp,
        ):
            xt_sb = constp.tile([128, NDC, R_CORE], f32r, tag="xt", name="xt_sb")
            qb_sb = constp.tile([128, 8, R_CORE], f32, tag="qb", name="qb_sb")
            qaug_sb = constp.tile([9, R_CORE], f32r, tag="qaug", name="qaug_sb")
            dwt_sb = constp.tile([128, NDC, 8], f32r, tag="dwt", name="dwt_sb")
            db_sb = constp.tile([8, 1], f32, tag="db", name="db_sb")
            baug_sb = constp.tile([9, NOC, 128], f32r, tag="baug", name="baug_sb")
            eye_sb = constp.tile([8, 8, 128], f32r, tag="eye", name="eye_sb")

            # input DMAs (per-partition rows contiguous in the host layout).
            # Order matters: the tiny tensors the decision GEMM + broadcasts
            # need come first so their transfers are not queued behind the
            # 4 MB of x chunks; j=0's first weights ride the same sync ring
            # right after x; everything else (34 weight blocks) goes on the
            # scalar engine's separate HWDGE ring.
            nc.sync.dma_start(out=dwt_sb[:, :, :], in_=dwt_d[:, :, :])
            nc.sync.dma_start(out=db_sb[:, :], in_=db_d[:, :])
            nc.sync.dma_start(out=eye_sb[:, :, :], in_=eye_d[:, :, :])
            for c in range(NDC):
                nc.sync.dma_start(out=xt_sb[:, c, :], in_=xt_d[:, c, :])
            nc.sync.dma_start(out=baug_sb[:, :, :], in_=baug_d[:, :, :])
            nc.sync.dma_start(out=qaug_sb[8:9, :], in_=ones_d[:, :])

            # 8 PSUM accumulator banks: out^T[oc*128:(oc+1)*128, rb*512:(rb+1)*512]
            acc = [
                [psp.tile([128, NB], f32, tag=f"acc{oc}{rb}", name=f"acc{oc}{rb}") for rb in range(NRB)]
                for oc in range(NOC)
            ]

            def main_mms(j, src, ocs):
                for oc in ocs:
                    w_sb = wp.tile([128, NDC, 128], f32r, tag="w", name="w_sb")
                    # j=0's first two weight blocks ride the sync HWDGE ring
                    # (issued right after x, before the scalar queue would get
                    # to them); the rest use the scalar engine's HWDGE ring so
                    # weight traffic does not serialize behind x on one ring.
                    weng = nc.sync if (j == 0 and oc >= 2) else nc.scalar
                    weng.dma_start(out=w_sb[:, :, :], in_=wt_d[j, oc])
                    for rb in range(NRB):
                        for c in range(NDC):
                            nc.tensor.matmul(
                                acc[oc][rb][:, :],
                                w_sb[:, c, :],
                                src[:, c, bass.ts(rb, NB)],
                                start=(j == 0 and c == 0),
                                stop=False,
                            )
                    if j == NJ - 1:
                        # bias GEMM closes the accumulation; drain + store
                        ost = ostp.tile([128, R_CORE], f32, tag="ost", name="ost")
                        for rb in range(NRB):
                            nc.tensor.matmul(
                                acc[oc][rb][:, :],
                                baug_sb[:, oc, :],
                                qaug_sb[:, bass.ts(rb, NB)],
                                start=False,
                                stop=True,
                            )
                            nc.vector.tensor_copy(ost[:, bass.ts(rb, NB)], acc[oc][rb][:, :])
                            nc.sync.dma_start(out=out_d[oc][:, bass.ts(rb, NB)], in_=ost[:, bass.ts(rb, NB)])

            # --- phase A1: decision GEMM + sigmoid -> qaug rows 0..7 ---
            for rb in range(NRB):
                dec_ps = acc[0][rb]
                for c in range(NDC):
                    nc.tensor.matmul(
                        dec_ps[0:8, :],
                        dwt_sb[:, c, :],
                        xt_sb[:, c, bass.ts(rb, NB)],
                        start=(c == 0),
                        stop=(c == NDC - 1),
                    )
                nc.scalar.activation(
                    qaug_sb[0:8, bass.ts(rb, NB)],
                    dec_ps[0:8, :],
                    mybir.ActivationFunctionType.Sigmoid,
                    bias=db_sb[0:8, 0:1],
                    scale=1.0,
                )

            # --- j=0 on output chunks 2,3 (keeps PE busy while sigmoid runs) ---
            main_mms(0, xt_sb, [2, 3])

            # --- phase A2: broadcast q_j to all partitions via one-hot outer
            # product; scratch banks acc[0..1][*] (not yet claimed by j=0) ---
            scratch = [acc[0][0], acc[0][1], acc[1][0], acc[1][1]]
            for j in range(8):
                for rb in range(NRB):
                    scr = scratch[(j * NRB + rb) % len(scratch)]
                    nc.tensor.matmul(
                        scr[:, :],
                        eye_sb[:, j, :],
                        qaug_sb[0:8, bass.ts(rb, NB)],
                        start=True,
                        stop=True,
                    )
                    nc.vector.tensor_copy(qb_sb[:, j, bass.ts(rb, NB)], scr[:, :])

            # --- j=0 on the remaining output chunks ---
            main_mms(0, xt_sb, [0, 1])

            # --- j=1..8: prescale x by q_{j-1}, then accumulate the delta GEMM ---
            for j in range(1, NJ):
                xs_sb = xsp.tile([128, NDC, R_CORE], f32r, tag="xs", name="xs_sb")
                for c in range(NDC):
                    nc.vector.tensor_tensor(
                        xs_sb[:, c, :],
                        xt_sb[:, c, :],
                        qb_sb[:, j - 1, :],
                        AluOpType.mult,
                    )
                main_mms(j, xs_sb, list(range(NOC)))

    return nc


def _get_nc():
    if "nc" not in _BUILD_CACHE:
        nc = _build_nc()
        nc.finalize()
        _BUILD_CACHE["nc"] = nc
    return _BUILD_CACHE["nc"]


def _prep_shared(dec_w, dec_b, leaf_w, leaf_b):
    leaf_w = np.asarray(leaf_w, np.float32)
    leaf_b = np.asarray(leaf_b, np.float32)
    dec_w = np.asarray(dec_w, np.float32)
    dec_b = np.asarray(dec_b, np.float32)

    w_odd = leaf_w[1::2]                         # [8, o, d]
    wcat = np.empty((NJ, D, D), np.float32)      # [j, o, d]
    wcat[0] = w_odd.sum(0)
    wcat[1:] = leaf_w[0::2] - w_odd
    wt_full = wcat.transpose(0, 2, 1)            # [j, d, o]

    # [NJ, NOC, 128(p), NDC(c), 128(o)]: stationary tile (j, oc, c) is
    # wt_full[j][c*128:(c+1)*128, oh*512 + oc*128 : +128]
    wt_halves = []
    for oh in range(N_O_HALVES):
        blk = wt_full[:, :, oh * O_CORE:(oh + 1) * O_CORE]      # [j, 1024(d), 512(o)]
        blk = blk.reshape(NJ, NDC, 128, NOC, 128)               # [j, c, p, oc, o]
        blk = blk.transpose(0, 3, 2, 1, 4)                      # [j, oc, p, c, o]
        wt_halves.append(np.ascontiguousarray(blk))

    dwt = np.ascontiguousarray(dec_w[7:15].T.reshape(NDC, 128, 8).transpose(1, 0, 2))
    db = np.ascontiguousarray(dec_b[7:15].reshape(8, 1))

    b_odd = leaf_b[1::2]
    baug = np.empty((9, D), np.float32)
    baug[0:8] = leaf_b[0::2] - b_odd
    baug[8] = b_odd.sum(0)
    baug_halves = [
        np.ascontiguousarray(baug[:, oh * O_CORE:(oh + 1) * O_CORE].reshape(9, NOC, 128))
        for oh in range(N_O_HALVES)
    ]

    eye8 = np.zeros((8, 8, 128), np.float32)
    eye8[np.arange(8), np.arange(8), :] = 1.0
    ones = np.ones((1, R_CORE), np.float32)
    return wt_halves, dwt, db, baug_halves, eye8, ones


def kernel(x, dec_w, dec_b, leaf_w, leaf_b):
    global LAST_RESULTS
    x = np.asarray(x, np.float32)
    wt_halves, dwt, db, baug_halves, eye8, ones = _prep_shared(dec_w, dec_b, leaf_w, leaf_b)

    xt = np.ascontiguousarray(x.reshape(R, D).T)    # [1024(d), 4096(r)]

    in_maps = []
    for core in range(N_CORES):
        rg, oh = divmod(core, N_O_HALVES)
        xt_core = np.ascontiguousarray(
            xt[:, rg * R_CORE:(rg + 1) * R_CORE]
            .reshape(NDC, 128, R_CORE)
            .transpose(1, 0, 2)
        )
        in_maps.append({
            "xt": xt_core,
            "wt": wt_halves[oh],
            "dwt": dwt,
            "db": db,
            "baug": baug_halves[oh],
            "eye8": eye8,
            "ones": ones,
        })

    nc = _get_nc()
    res = run_bass_kernel_spmd(nc, in_maps, core_ids=list(range(N_CORES)), **RUN_KWARGS)
    LAST_RESULTS = res

    out_t = np.empty((D, R), np.float32)
    for core in range(N_CORES):
        rg, oh = divmod(core, N_O_HALVES)
        o = res.results[core]["out"]      # [NOC, 128, R_CORE]
        for oc in range(NOC):
            out_t[oh * O_CORE + oc * 128: oh * O_CORE + (oc + 1) * 128,
                  rg * R_CORE:(rg + 1) * R_CORE] = o[oc]
    return np.ascontiguousarray(out_t.T).reshape(B, S, D)


# revision 7
# speedup vs baseline: 1.0226x; 1.0080x over previous
"""Trainium2 Bass kernel for the soft decision-tree MoE layer.

Math: with q_j = sigmoid(x . dec_w[7+j] + dec_b[7+j]) for j=0..7 (only the
last level of decision nodes feeds the leaves), the reference output is

    y = sum_l p_l * (x @ W_l^T + b_l),   p_{2j} = q_j, p_{2j+1} = 1 - q_j

which collapses to 9 GEMMs instead of 16:

    y = x @ W_base^T + sum_j (q_j * x) @ dW_j^T + Baug^T @ [q; 1]

    W_base = sum_j W_{2j+1},  dW_j = W_{2j} - W_{2j+1}
    Baug rows 0..7 = b_{2j} - b_{2j+1}, row 8 = sum_j b_{2j+1}

All 9 GEMMs + the bias GEMM accumulate in PSUM (out^T layout: outputs on
partitions, rows on the moving free dim). The per-row scale q_j * x is done
on the VectorE against a PE-broadcast copy of q_j (outer product with a
one-hot stationary operand, since engines cannot read partition-offset
APs). Matmuls run in float32r (fp22 multiply, fp32 accumulate), which
streams at full PE rate for free dim >= 256.

Sharding over 8 cores: 4 row groups (1024 rows each) x 2 output halves
(512 outs each). No cross-core communication; host assembles the slabs.
"""

import numpy as np

import concourse.bass as bass
import concourse.bacc as bacc
import concourse.tile as tile
from concourse import mybir
from concourse.alu_op_type import AluOpType
from concourse.bass_utils import run_bass_kernel_spmd

f32 = mybir.dt.float32
f32r = mybir.dt.float32r

B, S, D = 2, 2048, 1024
R = B * S                  # 4096 rows total
NJ = 9                     # W_base + 8 deltas
NDC = D // 128             # 8 contraction chunks
N_ROW_GROUPS = 4
N_O_HALVES = 2
N_CORES = N_ROW_GROUPS * N_O_HALVES
R_CORE = R // N_ROW_GROUPS         # 1024 rows per core
O_CORE = D // N_O_HALVES           # 512 outputs per core
NOC = O_CORE // 128                # 4 output chunks per core
NB = 512                           # moving-block (fp32 max free dim)
NRB = R_CORE // NB                 # 2 row blocks per core

# run options that test.py may override (e.g. trace=True)
RUN_KWARGS = {}
LAST_RESULTS = None

_BUILD_CACHE = {}


def _build_nc():
    nc = bacc.Bacc(None)

    xt_d = nc.dram_tensor("xt", [128, NDC, R_CORE], f32r, kind="ExternalInput")
    wt_d = nc.dram_tensor("wt", [NJ, NOC, 128, NDC, 128], f32r, kind="ExternalInput")
    dwt_d = nc.dram_tensor("dwt", [128, NDC, 8], f32r, kind="ExternalInput")
    db_d = nc.dram_tensor("db", [8, 1], f32, kind="ExternalInput")
    baug_d = nc.dram_tensor("baug", [9, NOC, 128], f32r, kind="ExternalInput")
    eye_d = nc.dram_tensor("eye8", [8, 8, 128], f32r, kind="ExternalInput")
    ones_d = nc.dram_tensor("ones", [1, R_CORE], f32r, kind="ExternalInput")
    out_d = nc.dram_tensor("out", [NOC, 128, R_CORE], f32, kind="ExternalOutput")

    with tile.TileContext(nc) as tc:
        with (
            tc.tile_pool(name="const", bufs=1) as constp,
            tc.tile_pool(name="xsp", bufs=2) as xsp,
            tc.tile_pool(name="wp", bufs=8) as wp,
            tc.tile_pool(name="ostp", bufs=2) as ostp,
            tc.tile_pool(name="psp", bufs=1, space="PSUM") as psp,
        ):
            xt_sb = constp.tile([128, NDC, R_CORE], f32r, tag="xt", name="xt_sb")
            qb_sb = constp.tile([128, 8, R_CORE], f32, tag="qb", name="qb_sb")
            qaug_sb = constp.tile([9, R_CORE], f32r, tag="qaug", name="qaug_sb")
            dwt_sb = constp.tile([128, NDC, 8], f32r, tag="dwt", name="dwt_sb")
            db_sb = constp.tile([8, 1], f32, tag="db", name="db_sb")
            baug_sb = constp.tile([9, NOC, 128], f32r, tag="baug", name="baug_sb")
            eye_sb = constp.tile([8, 8, 128], f32r, tag="eye", name="eye_sb")

            # input DMAs (per-partition rows contiguous in the host layout).
            # Order matters: the tiny tensors the decision GEMM + broadcasts
            # need come first so their transfers are not queued behind the
            # 4 MB of x chunks; j=0's first weights ride the same sync ring
            # right after x; everything else (34 weight blocks) goes on the
            # scalar engine's separate HWDGE ring.
            nc.sync.dma_start(out=dwt_sb[:, :, :], in_=dwt_d[:, :, :])
            nc.sync.dma_start(out=db_sb[:, :], in_=db_d[:, :])
            nc.sync.dma_start(out=eye_sb[:, :, :], in_=eye_d[:, :, :])
            for c in range(NDC):
                nc.sync.dma_start(out=xt_sb[:, c, :], in_=xt_d[:, c, :])
            nc.sync.dma_start(out=baug_sb[:, :, :], in_=baug_d[:, :, :])
            nc.sync.dma_start(out=qaug_sb[8:9, :], in_=ones_d[:, :])

            # 8 PSUM accumulator banks: out^T[oc*128:(oc+1)*128, rb*512:(rb+1)*512]
            acc = [
                [psp.tile([128, NB], f32, tag=f"acc{oc}{rb}", name=f"acc{oc}{rb}") for rb in range(NRB)]
                for oc in range(NOC)
            ]

            def main_mms(j, src, ocs):
                for oc in ocs:
                    w_sb = wp.tile([128, NDC, 128], f32r, tag="w", name="w_sb")
                    # j=0's first two weight blocks ride the sync HWDGE ring
                    # (issued right after x, before the scalar queue would get
                    # to them); the rest use the scalar engine's HWDGE ring so
                    # weight traffic does not serialize behind x on one ring.
                    weng = nc.sync if (j == 0 and oc >= 2) else nc.scalar
                    weng.dma_start(out=w_sb[:, :, :], in_=wt_d[j, oc])
                    for rb in range(NRB):
                        for c in range(NDC):
                            nc.tensor.matmul(
                                acc[oc][rb][:, :],
                                w_sb[:, c, :],
                                src[:, c, bass.ts(rb, NB)],
                                start=(j == 0 and c == 0),
                                stop=False,
                            )
                    if j == NJ - 1:
                        # bias GEMM closes the accumulation; drain + store
                        ost = ostp.tile([128, R_CORE], f32, tag="ost", name="ost")
                        for rb in range(NRB):
                            nc.tensor.matmul(
                                acc[oc][rb][:, :],
                                baug_sb[:, oc, :],
                                qaug_sb[:, bass.ts(rb, NB)],
                                start=False,
                                stop=True,
                            )
                            nc.vector.tensor_copy(ost[:, bass.ts(rb, NB)], acc[oc][rb][:, :])
                            nc.sync.dma_start(out=out_d[oc][:, bass.ts(rb, NB)], in_=ost[:, bass.ts(rb, NB)])

            # --- warmup: dummy matmuls on already-landed constants fill the
            # PE's DMA-wait window and trip the HAM clock gate to 2.4 GHz
            # before real work arrives (PE is otherwise idle here) ---
            for _ in range(12):
                nc.tensor.matmul(
                    acc[3][1][:, :],
                    eye_sb[:, 0, :],
                    eye_sb[:, 0:4, :],
                    start=True,
                    stop=True,
                )

            # --- phase A1: decision GEMM + sigmoid -> qaug rows 0..7 ---
            for rb in range(NRB):
                dec_ps = acc[0][rb]
                for c in range(NDC):
                    nc.tensor.matmul(
                        dec_ps[0:8, :],
                        dwt_sb[:, c, :],
                        xt_sb[:, c, bass.ts(rb, NB)],
                        start=(c == 0),
                        stop=(c == NDC - 1),
                    )
                nc.scalar.activation(
                    qaug_sb[0:8, bass.ts(rb, NB)],
                    dec_ps[0:8, :],
                    mybir.ActivationFunctionType.Sigmoid,
                    bias=db_sb[0:8, 0:1],
                    scale=1.0,
                )

            # --- j=0 on output chunks 2,3 (keeps PE busy while sigmoid runs) ---
            main_mms(0, xt_sb, [2, 3])

            # --- phase A2: broadcast q_j to all partitions via one-hot outer
            # product; scratch banks acc[0..1][*] (not yet claimed by j=0) ---
            scratch = [acc[0][0], acc[0][1], acc[1][0], acc[1][1]]
            for j in range(8):
                for rb in range(NRB):
                    scr = scratch[(j * NRB + rb) % len(scratch)]
                    nc.tensor.matmul(
                        scr[:, :],
                        eye_sb[:, j, :],
                        qaug_sb[0:8, bass.ts(rb, NB)],
                        start=True,
                        stop=True,
                    )
                    nc.vector.tensor_copy(qb_sb[:, j, bass.ts(rb, NB)], scr[:, :])

            # --- j=0 on the remaining output chunks ---
            main_mms(0, xt_sb, [0, 1])

            # --- j=1..8: prescale x by q_{j-1}, then accumulate the delta GEMM ---
            for j in range(1, NJ):
                xs_sb = xsp.tile([128, NDC, R_CORE], f32r, tag="xs", name="xs_sb")
                for c in range(NDC):
                    nc.vector.tensor_tensor(
                        xs_sb[:, c, :],
                        xt_sb[:, c, :],
                        qb_sb[:, j - 1, :],
                        AluOpType.mult,
                    )
                main_mms(j, xs_sb, list(range(NOC)))

    return nc


def _get_nc():
    if "nc" not in _BUILD_CACHE:
        nc = _build_nc()
        nc.finalize()
        _BUILD_CACHE["nc"] = nc
    return _BUILD_CACHE["nc"]


def _prep_shared(dec_w, dec_b, leaf_w, leaf_b):
    leaf_w = np.asarray(leaf_w, np.float32)
    leaf_b = np.asarray(leaf_b, np.float32)
    dec_w = np.asarray(dec_w, np.float32)
    dec_b = np.asarray(dec_b, np.float32)

    w_odd = leaf_w[1::2]                         # [8, o, d]
    wcat = np.empty((NJ, D, D), np.float32)      # [j, o, d]
    wcat[0] = w_odd.sum(0)
    wcat[1:] = leaf_w[0::2] - w_odd
    wt_full = wcat.transpose(0, 2, 1)            # [j, d, o]

    # [NJ, NOC, 128(p), NDC(c), 128(o)]: stationary tile (j, oc, c) is
    # wt_full[j][c*128:(c+1)*128, oh*512 + oc*128 : +128]
    wt_halves = []
    for oh in range(N_O_HALVES):
        blk = wt_full[:, :, oh * O_CORE:(oh + 1) * O_CORE]      # [j, 1024(d), 512(o)]
        blk = blk.reshape(NJ, NDC, 128, NOC, 128)               # [j, c, p, oc, o]
        blk = blk.transpose(0, 3, 2, 1, 4)                      # [j, oc, p, c, o]
        wt_halves.append(np.ascontiguousarray(blk))

    dwt = np.ascontiguousarray(dec_w[7:15].T.reshape(NDC, 128, 8).transpose(1, 0, 2))
    db = np.ascontiguousarray(dec_b[7:15].reshape(8, 1))

    b_odd = leaf_b[1::2]
    baug = np.empty((9, D), np.float32)
    baug[0:8] = leaf_b[0::2] - b_odd
    baug[8] = b_odd.sum(0)
    baug_halves = [
        np.ascontiguousarray(baug[:, oh * O_CORE:(oh + 1) * O_CORE].reshape(9, NOC, 128))
        for oh in range(N_O_HALVES)
    ]

    eye8 = np.zeros((8, 8, 128), np.float32)
    eye8[np.arange(8), np.arange(8), :] = 1.0
    ones = np.ones((1, R_CORE), np.float32)
    return wt_halves, dwt, db, baug_halves, eye8, ones


def kernel(x, dec_w, dec_b, leaf_w, leaf_b):
    global LAST_RESULTS
    x = np.asarray(x, np.float32)
    wt_halves, dwt, db, baug_halves, eye8, ones = _prep_shared(dec_w, dec_b, leaf_w, leaf_b)

    xt = np.ascontiguousarray(x.reshape(R, D).T)    # [1024(d), 4096(r)]

    in_maps = []
    for core in range(N_CORES):
        rg, oh = divmod(core, N_O_HALVES)
        xt_core = np.ascontiguousarray(
            xt[:, rg * R_CORE:(rg + 1) * R_CORE]
            .reshape(NDC, 128, R_CORE)
            .transpose(1, 0, 2)
        )
        in_maps.append({
            "xt": xt_core,
            "wt": wt_halves[oh],
            "dwt": dwt,
            "db": db,
            "baug": baug_halves[oh],
            "eye8": eye8,
            "ones": ones,
        })

    nc = _get_nc()
    res = run_bass_kernel_spmd(nc, in_maps, core_ids=list(range(N_CORES)), **RUN_KWARGS)
    LAST_RESULTS = res

    out_t = np.empty((D, R), np.float32)
    for core in range(N_CORES):
        rg, oh = divmod(core, N_O_HALVES)
        o = res.results[core]["out"]      # [NOC, 128, R_CORE]
        for oc in range(NOC):
            out_t[oh * O_CORE + oc * 128: oh * O_CORE + (oc + 1) * 128,
                  rg * R_CORE:(rg + 1) * R_CORE] = o[oc]
    return np.ascontiguousarray(out_t.T).reshape(B, S, D)
